# revision 11
# baseline (speedup 1.0000x reference)
"""Trainium2 Bass kernel v3 for nn_CrossAttention (B=2, Tq=Tk=2048, D=1024, H=16).

Sharding: 8 cores; core c owns batch b = c // 4 and query rows
[512*(c%4), 512*(c%4+1)). Each core computes full attention for its
query slice (all 16 heads). Unshard is a pure concat; no collectives.

All three input projections (Q, K, V) run as fp8e4m3 DoubleRow matmuls
with hi/lo splitting for ~bf16 accuracy at 0.75x the bf16 PE cost:
  x ~ xh + xl,  W*16 ~ Wh + Wl  (W scaled into fp8's normal range),
  x@W ~ (xh@Wh + xl@Wh + xh@Wl) / 16   [lo*lo dropped, ~bf16 level]
Each DoubleRow instruction computes two of these partial products
(paired across adjacent 128-deep contraction chunks) at 0.5 cyc/row.

The projected Q^T/K^T are quantized (single) fp8 in a [32-partition
block, half] layout so the scores matmul is fp8 DoubleRow as well
(64-dim head contraction = two 32-dim k-tiles). Softmax smooths the
quantization noise; V and P stay bf16 (direct paths).

attn@V is packed out[q,65]: P chunk stationary [128k,128q], V+ones
moving (N=65) -> full PE utilization; O^T recovered via PE transposes.

exp(P): 2.5:1.5 split between Act (native Exp from PSUM) and a DVE
copy -> Pool(gpsimd) pow(e,s) chain; attn@V software-pipelined 3 deep
behind the scores so producer latency never stalls the PE.

Biases fold into PE rank-1 matmuls or the PSUM->SBUF quantize copies.
V projection interleaved per key-chunk; 4 rounds x 4 heads so the 16
packed 65-col fp32 accumulators fit 3 PSUM banks (one start=True per
bank; lazy pending-zero covers the other slots).
"""

import numpy as np
import ml_dtypes

import concourse.bass as bass
import concourse.mybir as mybir
import concourse.tile as tile
from concourse import bacc
from concourse.bass_utils import run_bass_kernel_spmd
from concourse.bass_interp import get_hw_module

B, TQ, TK, D, H = 2, 2048, 2048, 1024, 16
HD = D // H  # 64
N_CORES = 8
QLOC = (B * TQ) // N_CORES  # 512
S8 = float(HD ** -0.25)     # per-side score scale; S8^2 = 1/8
WS = 16.0                   # weight pre-scale for fp8 hi/lo quantization

F32 = mybir.dt.float32
BF16 = mybir.dt.bfloat16
F8 = mybir.dt.float8e4
DR = mybir.MatmulPerfMode.DoubleRow
Exp = mybir.ActivationFunctionType.Exp
Ident = mybir.ActivationFunctionType.Identity
Copy = mybir.ActivationFunctionType.Copy
ADD = mybir.AluOpType.add
MULT = mybir.AluOpType.mult
POW = mybir.AluOpType.pow

_cache: dict = {}

FP8T = ml_dtypes.float8_e4m3


def _perm1024():
    """Permuted col c*128 + 32*g + r <- head-dim (4*(c//2)+g)*64 +
    (c%2)*32 + r, so fp8 Q^T/K^T land in the [32-block, half] layout
    the DoubleRow scores matmul needs."""
    perm = np.empty(D, np.int64)
    for c in range(8):
        j, half = c // 2, c % 2
        for g in range(4):
            for r in range(32):
                perm[c * 128 + 32 * g + r] = (4 * j + g) * 64 + half * 32 + r
    return perm


PERM = _perm1024()


def _build_v3(n_kc: int, masked: tuple):
    NK = n_kc * 128
    nsp = [(s, min(512, NK - s)) for s in range(0, NK, 512)]

    nc = bacc.Bacc("TRN2", target_bir_lowering=False, debug=False,
                   num_devices=N_CORES)

    # hi/lo fp8 inputs: x* = activations (scale 1), w* = weights (x WS)
    xqh_d = nc.dram_tensor("xqh", [128, 4, 2, QLOC], F8, kind="ExternalInput")
    xql_d = nc.dram_tensor("xql", [128, 4, 2, QLOC], F8, kind="ExternalInput")
    wqh_d = nc.dram_tensor("wqh", [128, 4, 2, D], F8, kind="ExternalInput")
    wkh_d = nc.dram_tensor("wkh", [128, 4, 2, D], F8, kind="ExternalInput")
    xkh_d = nc.dram_tensor("xkh", [128, 4, 2, NK], F8, kind="ExternalInput")
    xkl_d = nc.dram_tensor("xkl", [128, 4, 2, NK], F8, kind="ExternalInput")
    wvh_d = nc.dram_tensor("wvh", [128, 4, 2, D], F8, kind="ExternalInput")
    wvl_d = nc.dram_tensor("wvl", [128, 4, 2, D], F8, kind="ExternalInput")
    wo16_d = nc.dram_tensor("wo16", [128, 8, D], BF16, kind="ExternalInput")
    bq_d = nc.dram_tensor("bq_s", [128, 8], F32, kind="ExternalInput")
    bk_d = nc.dram_tensor("bk_s", [128, 8], F32, kind="ExternalInput")
    bv_d = nc.dram_tensor("bv_row", [1, D], F32, kind="ExternalInput")
    bo_d = nc.dram_tensor("bo_row", [1, D], F32, kind="ExternalInput")
    bias_d = nc.dram_tensor("biask_row", [1, NK], F32, kind="ExternalInput")
    id_d = nc.dram_tensor("ident", [128, 128], BF16, kind="ExternalInput")
    y_d = nc.dram_tensor("y", [QLOC, D], BF16, kind="ExternalOutput")

    with tile.TileContext(nc) as tc:
        with (
            tc.tile_pool(name="const", bufs=1) as const,
            tc.tile_pool(name="persist", bufs=1) as persist,
            tc.tile_pool(name="work", bufs=4) as work,
            tc.tile_pool(name="scp", bufs=4) as scp,
            tc.tile_pool(name="rbp", bufs=6) as rbp,
        ):
            # ---------- persistent ----------
            qp8 = persist.tile([128, 4, 2, QLOC], F8)
            kp8 = persist.tile([128, 4, 2, NK], F8)
            v_sb = persist.tile([128, n_kc, H * 65], BF16)
            ot = persist.tile([128, 4, D], BF16)
            otT = persist.tile([128, 8, QLOC], BF16)
            wo_sb = persist.tile([128, 8, D], BF16)

            # const tiles (DMAs emitted after stage-A inputs, below)
            ones = const.tile([1, 512], F32)
            e_t = const.tile([128, 512], F32)
            bq_sb = const.tile([128, 8], F32)
            bk_sb = const.tile([128, 8], F32)
            bv_bc = const.tile([128, D], F32)
            bo_bc = const.tile([128, D], F32)
            bias_sb = const.tile([1, NK], F32)
            id_sb = const.tile([128, 128], BF16)

            with (
                tc.tile_pool(name="ld8", bufs=1) as ld8,
                tc.tile_pool(name="psC", bufs=1, space="PSUM") as psC,
                tc.tile_pool(name="pss", bufs=4, space="PSUM") as pssp,
                tc.tile_pool(name="po", bufs=3, space="PSUM") as pop,
                tc.tile_pool(name="ptp", bufs=10) as ptp,
            ):
                def dp_tiles(name, n):
                    t = ld8.tile([128, 4, 2, n], F8, name=name)
                    return t, [t[:, d] for d in range(4)]

                xqh_t, xqh = dp_tiles("xqh", QLOC)
                xql_t, xql = dp_tiles("xql", QLOC)
                wqh_t, wqh = dp_tiles("wqh", D)
                wkh_t, wkh = dp_tiles("wkh", D)
                xkh_t, xkh = dp_tiles("xkh", NK)
                xkl_t, xkl = dp_tiles("xkl", NK)
                wvh_t, wvh = dp_tiles("wvh", D)
                wvl_t, wvl = dp_tiles("wvl", D)

                # --- DMAs: stage-A inputs first; wqh/xqh split so the
                # first matmuls start ~2.5us earlier ---
                nc.sync.dma_start(wqh_t[:, :, :, 0:256],
                                  wqh_d.ap()[:, :, :, 0:256])
                nc.sync.dma_start(xqh_t[:, 0:2], xqh_d.ap()[:, 0:2])
                nc.sync.dma_start(wqh_t[:, :, :, 256:512],
                                  wqh_d.ap()[:, :, :, 256:512])
                nc.sync.dma_start(xqh_t[:, 2:4], xqh_d.ap()[:, 2:4])
                nc.sync.dma_start(wqh_t[:, :, :, 512:1024],
                                  wqh_d.ap()[:, :, :, 512:1024])
                nc.sync.dma_start(xql_t[:], xql_d.ap())
                # consts (small), then B inputs, kv col-groups, V weights
                nc.vector.memset(ones[:], 1.0)
                nc.vector.memset(e_t[:], float(np.e))
                nc.sync.dma_start(bq_sb[:], bq_d.ap())
                nc.sync.dma_start(bk_sb[:], bk_d.ap())
                nc.sync.dma_start(bv_bc[0:1, :], bv_d.ap())
                nc.gpsimd.partition_broadcast(bv_bc[:], bv_bc[0:1, :])
                nc.sync.dma_start(bo_bc[0:1, :], bo_d.ap())
                nc.gpsimd.partition_broadcast(bo_bc[:], bo_bc[0:1, :])
                nc.sync.dma_start(bias_sb[:], bias_d.ap())
                nc.sync.dma_start(id_sb[:], id_d.ap())
                nc.sync.dma_start(wkh_t[:], wkh_d.ap())
                for gi, (s, w) in enumerate(nsp):
                    nc.sync.dma_start(xkh_t[:, :, :, s:s + w],
                                      xkh_d.ap()[:, :, :, s:s + w])
                    nc.sync.dma_start(xkl_t[:, :, :, s:s + w],
                                      xkl_d.ap()[:, :, :, s:s + w])
                    if gi == 0:
                        nc.sync.dma_start(wvh_t[:], wvh_d.ap())
                        nc.sync.dma_start(wvl_t[:], wvl_d.ap())
                nc.sync.dma_start(wo_sb[:], wo16_d.ap())

                # ones columns of v_sb
                v_view = v_sb[:].rearrange("p k (h c) -> p k h c", c=65)
                nc.vector.memset(v_view[:, :, :, 64:65], 1.0)

                def hl_group(ps, prods, cols, rhs_sl, width):
                    """DoubleRow matmul sweep over (W, x) product pairs."""
                    for pi, (wt, xt) in enumerate(prods):
                        for dp in range(4):
                            nc.tensor.matmul(
                                ps[:, :width], wt[dp][:, :, cols],
                                xt[dp][:, :, rhs_sl],
                                start=(pi == 0 and dp == 0),
                                stop=(pi == len(prods) - 1 and dp == 3),
                                perf_mode=DR,
                            )

                # ---------- stage A: Q^T projection ----------
                for c in range(8):
                    ps = pssp.tile([128, QLOC], F32, tag="pss")
                    hl_group(ps, ((wqh, xqh), (wqh, xql)),
                             bass.ts(c, 128), slice(0, QLOC), QLOC)
                    dst = qp8[:, c // 2, c % 2, :]
                    if c % 2 == 0:
                        nc.scalar.activation(dst, ps[:], Ident,
                                             bias=bq_sb[:, c:c + 1],
                                             scale=S8 / WS)
                    else:
                        nc.vector.tensor_scalar(
                            out=dst, in0=ps[:], scalar1=S8 / WS,
                            scalar2=bq_sb[:, c:c + 1], op0=MULT, op1=ADD)

                # ---------- stage B: K^T projection (col-group major) ------
                def emit_B_group(si):
                    s, w = nsp[si]
                    for c in range(8):
                        ps = pssp.tile([128, QLOC], F32, tag="pss")
                        hl_group(ps, ((wkh, xkh), (wkh, xkl)),
                                 bass.ts(c, 128), slice(s, s + w), w)
                        dst = kp8[:, c // 2, c % 2, s:s + w]
                        if (c + si) % 2 == 0:
                            nc.scalar.activation(dst, ps[:, :w], Ident,
                                                 bias=bk_sb[:, c:c + 1],
                                                 scale=S8 / WS)
                        else:
                            nc.vector.tensor_scalar(
                                out=dst, in0=ps[:, :w], scalar1=S8 / WS,
                                scalar2=bk_sb[:, c:c + 1], op0=MULT, op1=ADD)

                # interleave later B groups into stage D's first kc
                # iterations when the kc range allows (kc 0-3 only need
                # group 0); otherwise emit everything upfront.
                interleave_B = n_kc >= 4 * len(nsp) - 2 and len(nsp) > 1
                emit_B_group(0)
                if not interleave_B:
                    for si in range(1, len(nsp)):
                        emit_B_group(si)

                # ---------- stage C (V projection quarters) ----------
                def emit_C(kc, dvq):
                    ps = psC.tile([128, 256], F32, tag="psc")
                    pi = 0
                    for (wt, xt) in ((wvh, xkh), (wvh, xkl), (wvl, xkh)):
                        for dp in range(4):
                            nc.tensor.matmul(
                                ps[:], xt[dp][:, :, bass.ts(kc, 128)],
                                wt[dp][:, :, bass.ts(dvq, 256)],
                                start=(pi == 0), stop=(pi == 11),
                                perf_mode=DR,
                            )
                            pi += 1
                    dstv = v_view[:, kc, 4 * dvq:4 * dvq + 4, 0:64]
                    srcv = ps[:].rearrange("p (h c) -> p h c", c=64)
                    bvv = bv_bc[:, bass.ts(dvq, 256)].rearrange(
                        "p (h c) -> p h c", c=64)
                    nc.vector.scalar_tensor_tensor(
                        out=dstv, in0=srcv, scalar=1.0 / WS, in1=bvv,
                        op0=MULT, op1=ADD)

                # ---------- stage D: 4 rounds x 4 heads ----------
                SLOT_W = 65

                def slot_ap(po_banks, s):
                    b, j = s // 7, s % 7
                    return po_banks[b][:, SLOT_W * j:SLOT_W * j + SLOT_W]

                SLOT_W = 65
                po_banks = {}   # hg -> [3 tiles], allocated lazily
                pending = []

                def slot_ap(hg0, s):
                    b, j = s // 7, s % 7
                    return po_banks[hg0][b][:, SLOT_W * j:SLOT_W * j + SLOT_W]

                def emit_norm(hg0):
                    rbs = []
                    for b in range(3):
                        lo = 7 * b
                        ns = min(7, 16 - lo)
                        rb = rbp.tile([128, 7], F32, tag="rb",
                                      name=f"rb_{hg0}_{b}")
                        col64 = po_banks[hg0][b][:, 0:SLOT_W * ns].rearrange(
                            "p (s c) -> p s c", c=65)[:, :, 64]
                        nc.vector.reciprocal(rb[:, 0:ns], col64)
                        rbs.append(rb)
                    for s in range(16):
                        hh, u = s // 4, s % 4
                        h = 4 * hg0 + hh
                        sl = 4 * hh + u
                        num = slot_ap(hg0, sl)[:, 0:64]
                        rb1 = rbs[sl // 7][:, sl % 7:sl % 7 + 1]
                        dst = ot[:, u, 64 * h:64 * h + 64]
                        if s % 2 == 0:
                            nc.scalar.activation(dst, num, Copy, scale=rb1)
                        else:
                            nc.vector.tensor_scalar_mul(dst, num, rb1)

                def flush_attnv():
                    hg0, kc0, hh0, pt0 = pending.pop(0)
                    if hg0 not in po_banks:
                        po_banks[hg0] = [
                            pop.tile([128, 512], F32, tag="po",
                                     name=f"po_{hg0}_{b}") for b in range(3)]
                    h0 = 4 * hg0 + hh0
                    for u in range(4):
                        s = 4 * hh0 + u
                        st = (kc0 == 0) and (s % 7 == 0)
                        nc.tensor.matmul(
                            slot_ap(hg0, s),
                            pt0[:, bass.ts(u, 128)],
                            v_view[:, kc0, h0, :],
                            start=st, stop=(kc0 == n_kc - 1),
                            skip_group_check=True,
                        )
                    if kc0 == n_kc - 1 and hh0 == 3:
                        emit_norm(hg0)

                for hg in range(4):
                    emit_C(0, hg)
                    if n_kc > 1:
                        emit_C(1, hg)
                    for kc in range(n_kc):
                        for hh in range(4):
                            h = 4 * hg + hh
                            j, g = h // 4, h % 4
                            pss = pssp.tile([128, QLOC], F32, tag="pss")
                            has_bias = kc in masked
                            nc.tensor.matmul(
                                pss[:],
                                kp8[32 * g:32 * g + 32, j, :, bass.ts(kc, 128)],
                                qp8[32 * g:32 * g + 32, j, :, :],
                                start=True, stop=not has_bias, perf_mode=DR,
                                tile_position=(32 * g, 0),
                            )
                            if has_bias:
                                nc.tensor.matmul(
                                    pss[:], bias_sb[:, bass.ts(kc, 128)],
                                    ones[:, 0:QLOC],
                                    start=False, stop=True,
                                    skip_group_check=True,
                                )
                            if hh == 0 and kc + 2 < n_kc:
                                emit_C(kc + 2, hg)
                            # P production, 2.5 Act : 1.5 chain
                            pt = ptp.tile([128, QLOC], BF16, tag="pt")
                            use_act = hh < 2 or (hh == 2 and kc % 2 == 0)
                            if use_act:
                                nc.scalar.activation(pt[:], pss[:], Exp)
                            else:
                                sc = scp.tile([128, QLOC], F32, tag="sc")
                                nc.vector.tensor_copy(sc[:], pss[:])
                                nc.gpsimd.tensor_tensor(
                                    out=pt[:], in0=e_t[:], in1=sc[:], op=POW)
                            pending.append((hg, kc, hh, pt))
                            if len(pending) > 6:
                                flush_attnv()
                        if interleave_B and hg == 0 and 1 <= kc < len(nsp):
                            emit_B_group(kc)
                while pending:
                    flush_attnv()

            # ---------- stage E: transpose + output projection ----------
            with (
                tc.tile_pool(name="stp", bufs=3, space="PSUM") as stp,
                tc.tile_pool(name="psE", bufs=2, space="PSUM") as psE,
            ):
                for u in range(4):
                    for dc in range(8):
                        tp = stp.tile([128, 128], BF16, tag="tp")
                        nc.tensor.transpose(
                            tp[:], ot[:, u, bass.ts(dc, 128)], id_sb[:])
                        dst = otT[:, dc, bass.ts(u, 128)]
                        if dc % 2 == 0:
                            nc.scalar.activation(dst, tp[:], Copy)
                        else:
                            nc.vector.tensor_copy(dst, tp[:])
                for u in range(4):
                    for nn in range(2):
                        ps = psE.tile([128, 512], F32, tag="pse")
                        for mc in range(8):
                            nc.tensor.matmul(
                                ps[:], otT[:, mc, bass.ts(u, 128)],
                                wo_sb[:, mc, bass.ts(nn, 512)],
                                start=(mc == 0), stop=(mc == 7),
                            )
                        y_sb = work.tile([128, 512], BF16, tag="y")
                        nc.vector.tensor_tensor(
                            out=y_sb[:], in0=ps[:],
                            in1=bo_bc[:, bass.ts(nn, 512)], op=ADD)
                        nc.sync.dma_start(
                            y_d.ap()[bass.ts(u, 128), bass.ts(nn, 512)],
                            y_sb[:])

    nc.compile()
    nc.m = get_hw_module(nc.m)
    return nc


def _get_program(n_kc: int, masked: tuple):
    key = (n_kc, masked)
    if key not in _cache:
        _cache[key] = _build_v3(n_kc, masked)
    return _cache[key]


def _bf(x):
    return np.ascontiguousarray(x).astype(ml_dtypes.bfloat16)


def _pack_dr(xt):
    """[1024(d), N] -> [128, 4, 2, N]: partition-major DR pairing."""
    n = xt.shape[1]
    return np.ascontiguousarray(
        xt.reshape(4, 2, 128, n).transpose(2, 0, 1, 3))


def _hl(x, scale=1.0):
    """hi/lo fp8 split of x*scale; returns fp8 arrays."""
    xs = np.asarray(x, np.float32) * scale
    h = xs.astype(FP8T)
    l = (xs - h.astype(np.float32)).astype(FP8T)
    return h, l


def kernel(q, kv, key_padding_mask, Wq, bq, Wkv, bkv, Wo, bo):
    q = np.asarray(q, dtype=np.float32)
    kv = np.asarray(kv, dtype=np.float32)
    mask = np.asarray(key_padding_mask).astype(bool)
    Wq = np.asarray(Wq, dtype=np.float32)
    bq = np.asarray(bq, dtype=np.float32)
    Wkv = np.asarray(Wkv, dtype=np.float32)
    bkv = np.asarray(bkv, dtype=np.float32)
    Wo = np.asarray(Wo, dtype=np.float32)
    bo = np.asarray(bo, dtype=np.float32)

    live = ~mask
    chunk_live = live.reshape(B, TK // 128, 128).any(axis=2).any(axis=0)
    active = np.flatnonzero(chunk_live)
    n_kc = int(len(active))
    assert n_kc >= 1
    NK = n_kc * 128
    sel = (active[:, None] * 128 + np.arange(128)[None, :]).reshape(-1)
    any_masked = mask[:, sel].any(axis=0).reshape(n_kc, 128).any(axis=1)
    masked = tuple(int(i) for i in np.flatnonzero(any_masked))

    nc = _get_program(n_kc, masked)

    Wk = Wkv[:, :D]
    Wv = Wkv[:, D:]
    bk = bkv[:D]
    bv = bkv[D:]
    wqh, _ = _hl(Wq[:, PERM], WS)
    wkh, _ = _hl(Wk[:, PERM], WS)
    wvh, wvl = _hl(Wv, WS)
    shared = {
        "wqh": _pack_dr(wqh),
        "wkh": _pack_dr(wkh),
        "wvh": _pack_dr(wvh), "wvl": _pack_dr(wvl),
        "wo16": np.ascontiguousarray(
            _bf(Wo).reshape(8, 128, D).transpose(1, 0, 2)),
        "bq_s": np.ascontiguousarray(
            (bq[PERM] * S8).reshape(8, 128).T).astype(np.float32),
        "bk_s": np.ascontiguousarray(
            (bk[PERM] * S8).reshape(8, 128).T).astype(np.float32),
        "bv_row": bv.reshape(1, D).astype(np.float32),
        "bo_row": bo.reshape(1, D).astype(np.float32),
        "ident": np.eye(128, dtype=np.float32).astype(ml_dtypes.bfloat16),
    }

    in_maps = []
    for c in range(N_CORES):
        b = c // 4
        r0 = (c % 4) * QLOC
        xqh, xql = _hl(q[b, r0:r0 + QLOC, :].T)
        kvt = kv[b][sel, :].T
        xkh, xkl = _hl(kvt)
        biask_row = np.where(mask[b][sel], np.float32(-80.0),
                             np.float32(0.0)).reshape(1, NK)
        m = dict(shared)
        m.update({"xqh": _pack_dr(xqh), "xql": _pack_dr(xql),
                  "xkh": _pack_dr(xkh), "xkl": _pack_dr(xkl),
                  "biask_row": biask_row})
        in_maps.append(m)

    res = run_bass_kernel_spmd(
        nc, in_maps, core_ids=list(range(N_CORES)), trace=False)

    out = np.empty((B, TQ, D), dtype=np.float32)
    for c in range(N_CORES):
        b = c // 4
        r0 = (c % 4) * QLOC
        out[b, r0:r0 + QLOC, :] = np.asarray(
            res.results[c]["y"]).astype(np.float32)
    return out
